# revision 1
# baseline (speedup 1.0000x reference)
"""Trainium2 Bass kernel for nn_CriticNetwork (3x GATConv + pool + MLP head).

Strategy (8-way graph/data parallel):
- Graphs are contiguous node ranges (batch is sorted). Core c owns graphs
  [8c, 8c+8) = nodes [ns_c, ne_c), and all edges whose dst lands in that
  range. Edges are sorted by dst and chopped into 128-edge tiles that never
  cross a 128-node "window"; per-window tiles accumulate into one PSUM bank
  via a rank-onehot scatter matmul (out[d] += onehot^T @ msgs).
- Layer 1 (1->256, rank-1 in x0) and the dynamic layer (3->64, rank-3 in
  x_dyn) collapse: per-edge messages are scalars ex*x[src], so pass A only
  gathers x[src] (16B) and aggregates 12 columns per window.
- Node phase expands aggregates to h1 (rank-1), applies elu, computes
  h2 = elu(h1) @ Ws2 and attention dot-products; slices are AllGathered so
  pass B can gather h2[src] rows (1KB) for the full-rank layer 2.
- Pooling is a per-window matmul with a host-built (1/count) mask; the tiny
  value head runs on-device per core over its 8 graphs.

kernel(**inputs) is self-contained: host-side work is only sharding
(partition/sort/pad of indices, slicing, dtype casts) — all model math,
including derived attention constants, runs on device.
"""

import numpy as np

import concourse.bacc as bacc
import concourse.bass as bass
import concourse.mybir as mybir
import concourse.tile as tile
from concourse.masks import make_identity

F32 = mybir.dt.float32
F32R = mybir.dt.float32r
I32 = mybir.dt.int32
AF = mybir.ActivationFunctionType
OP = mybir.AluOpType

P = 128
H = 4          # heads (static encoder)
C = 64         # channels per head
HC = H * C     # 256
KB = 8         # tiles batched per DVE macro-op group
EPS = 1e-16


def brd(ap, pattern, offset=None):
    """Manual broadcast: new AP over same tensor with given [step, count] list."""
    return bass.AP(ap.tensor, ap.offset if offset is None else offset, pattern)


# ----------------------------------------------------------------------------
# Host-side sharding / planning
# ----------------------------------------------------------------------------

class Plan:
    pass


def host_prep(x, edge_attr, edge_index, batch, n_graphs, n_cores):
    """Pure index/layout work (sharding); no model math."""
    N = x.shape[0]
    W = n_cores
    gpc = n_graphs // W  # graphs per core
    assert gpc * W == n_graphs

    batch = np.asarray(batch).astype(np.int64)
    src = np.asarray(edge_index[0]).astype(np.int64)
    dst = np.asarray(edge_index[1]).astype(np.int64)
    ea = np.asarray(edge_attr).astype(np.float32)
    x = np.asarray(x).astype(np.float32)

    node_start = np.searchsorted(batch, np.arange(n_graphs + 1))
    core_ns = node_start[0::gpc]  # [W+1] boundaries
    nk = np.diff(core_ns)
    R = int(128 * np.ceil(nk.max() / 128))
    nwin = R // 128
    NP = W * R

    core_of = np.searchsorted(core_ns, np.arange(N), side="right") - 1
    pid = core_of * R + (np.arange(N) - core_ns[core_of])
    x_pad = np.zeros((NP, 4), np.float32)
    x_pad[pid] = x

    counts = np.bincount(batch, minlength=n_graphs).astype(np.float32)
    assert (counts > 0).all()

    dcore = np.searchsorted(core_ns, dst, side="right") - 1

    # per-core sorted edge lists and window boundaries
    per_core = []
    for c in range(W):
        m = dcore == c
        dl = (dst[m] - core_ns[c]).astype(np.int64)
        order = np.argsort(dl, kind="stable")
        e_src = pid[src[m]][order].astype(np.int32)
        e_dl = dl[order].astype(np.int32)
        e_ea = ea[m][order]
        bounds = np.searchsorted(e_dl, np.arange(nwin + 1) * 128)
        per_core.append((e_src, e_dl, e_ea, bounds))

    tiles_per_window = []
    for w in range(nwin):
        mx = 1
        for c in range(W):
            b = per_core[c][3]
            mx = max(mx, int(np.ceil((b[w + 1] - b[w]) / 128)))
        tiles_per_window.append(mx)
    T = int(np.sum(tiles_per_window))

    per_core_arrays = []
    for c in range(W):
        e_src, e_dl, e_ea, bounds = per_core[c]
        msrc = np.zeros((T, P), np.int32)
        mdl = np.full((T, P), R, np.int32)
        mrank = np.full((T, P), P, np.int32)
        eat = np.zeros((T, P, 2), np.float32)
        ti = 0
        for w in range(nwin):
            e0, e1 = int(bounds[w]), int(bounds[w + 1])
            for j in range(tiles_per_window[w]):
                a = e0 + P * j
                b = min(a + P, e1)
                if b > a:
                    n = b - a
                    msrc[ti, :n] = e_src[a:b]
                    mdl[ti, :n] = e_dl[a:b]
                    mrank[ti, :n] = e_dl[a:b] - 128 * w
                    eat[ti, :n] = e_ea[a:b]
                ti += 1
        assert ti == T

        ns, ne = int(core_ns[c]), int(core_ns[c + 1])
        x_own = np.zeros((R, 4), np.float32)
        x_own[: ne - ns] = x[ns:ne]
        pmask = np.zeros((R, gpc), np.float32)
        gidx = (batch[ns:ne] - c * gpc).astype(np.int64)
        pmask[np.arange(ne - ns), gidx] = 1.0 / counts[batch[ns:ne]]

        per_core_arrays.append(
            dict(
                x_own=x_own,
                m_src=np.ascontiguousarray(msrc.T),          # [128, T]
                m_dl=np.ascontiguousarray(mdl.T),            # [128, T]
                m_rank=np.ascontiguousarray(mrank.T),        # [128, T]
                ea_t=np.ascontiguousarray(eat.transpose(1, 0, 2)),  # [128, T, 2]
                pmask=pmask,
            )
        )

    plan = Plan()
    plan.W = W
    plan.R = R
    plan.NP = NP
    plan.nwin = nwin
    plan.T = T
    plan.tiles_per_window = tiles_per_window
    plan.gpc = gpc
    return plan, x_pad, per_core_arrays


# ----------------------------------------------------------------------------
# Device program
# ----------------------------------------------------------------------------

def build_bass(plan):
    W, R, NP, nwin, T = plan.W, plan.R, plan.NP, plan.nwin, plan.T
    tpw = plan.tiles_per_window
    gpc = plan.gpc

    nc = bacc.Bacc("TRN2", target_bir_lowering=False, debug=False, num_devices=W)

    def dp(name, shape, dtype=F32, out=False):
        return nc.declare_dram_parameter(name, list(shape), dtype, isOutput=out)

    x_pad = dp("x_pad", [NP, 4])
    x_own = dp("x_own", [R, 4])
    m_src = dp("m_src", [P, T], I32)
    m_dl = dp("m_dl", [P, T], I32)
    m_rank = dp("m_rank", [P, T], I32)
    ea_in = dp("ea_t", [P, T, 2])
    pmask = dp("pmask", [R, gpc])

    ws1 = dp("ws1", [1, HC])
    a1s = dp("a1s", [1, HC])
    a1d = dp("a1d", [1, HC])
    we1 = dp("we1", [1, 2 * HC])
    ae1 = dp("ae1", [1, HC])
    bs1 = dp("bs1", [1, HC])
    ws2 = dp("ws2", [HC, HC])
    a2s = dp("a2s", [1, HC])
    a2d = dp("a2d", [1, HC])
    we2 = dp("we2", [1, 2 * HC])
    ae2 = dp("ae2", [1, HC])
    bs2 = dp("bs2", [1, C])
    wd = dp("wd", [3, C])
    wdf = dp("wdf", [1, 3 * C])
    ads = dp("ads", [1, C])
    add_ = dp("add", [1, C])
    bd = dp("bd", [1, C])
    wv1 = dp("wv1", [C, C])
    bv1 = dp("bv1", [1, C])
    wv2 = dp("wv2", [C, 1])
    bv2 = dp("bv2", [1, 1])

    v_out = dp("v", [gpc, 1], out=True)

    # internal DRAM
    dstvA = nc.dram_tensor("dstvA", [R + P, 5], F32)
    dstv2 = nc.dram_tensor("dstv2", [R + P, 4], F32)
    h2slice = nc.dram_tensor("h2slice", [R, 4 + HC], F32)
    if W > 4:
        H2ext = nc.dram_tensor("H2ext", [NP, 4 + HC], F32, addr_space="Shared")
    else:
        H2ext = nc.dram_tensor("H2ext", [NP, 4 + HC], F32)
    HR = 4 + HC  # gathered row width for pass B (h2 | s2src)

    with tile.TileContext(nc) as tc:
        with (
            tc.tile_pool(name="const", bufs=1) as cp,
            tc.tile_pool(name="meta", bufs=1) as mp,
            tc.tile_pool(name="work", bufs=3) as wp,
            tc.tile_pool(name="ps", bufs=4, space="PSUM") as pp,
            tc.tile_pool(name="pst", bufs=2, space="PSUM") as pt,
        ):
            # ---------------- P0: constants -------------------------------
            ident = cp.tile([P, P], F32)
            make_identity(nc, ident[:])
            iota_mat = cp.tile([P, P], I32)
            nc.gpsimd.iota(iota_mat[:], pattern=[[1, P]], base=0, channel_multiplier=0)

            def load_row(dram, width, tag):
                t = cp.tile([1, width], F32, tag=tag)
                nc.sync.dma_start(out=t[:], in_=dram[0:1, 0:width])
                return t

            r_ws1 = load_row(ws1, HC, "r_ws1")
            r_a1s = load_row(a1s, HC, "r_a1s")
            r_a1d = load_row(a1d, HC, "r_a1d")
            r_we1 = load_row(we1, 2 * HC, "r_we1")
            r_ae1 = load_row(ae1, HC, "r_ae1")
            r_bs1 = load_row(bs1, HC, "r_bs1")
            r_a2s = load_row(a2s, HC, "r_a2s")
            r_a2d = load_row(a2d, HC, "r_a2d")
            r_we2 = load_row(we2, 2 * HC, "r_we2")
            r_ae2 = load_row(ae2, HC, "r_ae2")
            r_bs2 = load_row(bs2, C, "r_bs2")
            r_wdf = load_row(wdf, 3 * C, "r_wdf")
            r_ads = load_row(ads, C, "r_ads")
            r_add = load_row(add_, C, "r_add")
            r_bd = load_row(bd, C, "r_bd")
            r_bv1 = load_row(bv1, C, "r_bv1")
            r_bv2 = load_row(bv2, 1, "r_bv2")

            scratch = cp.tile([1, 2 * HC], F32)

            def dot_heads(out_ap, wrow, arow, nh):
                """out[0, h] = sum_c wrow[0, h*C+c] * arow[0, h*C+c]."""
                nc.vector.tensor_tensor(
                    out=scratch[0:1, 0 : nh * C], in0=wrow, in1=arow, op=OP.mult
                )
                nc.vector.reduce_sum(
                    out=out_ap,
                    in_=brd(scratch[:], [scratch[:].ap[0], [C, nh], [1, C]]),
                    axis=mybir.AxisListType.X,
                )

            # cc = [c1(4) | c1d(4)]
            cc_row = cp.tile([1, 2 * H], F32)
            dot_heads(cc_row[0:1, 0:H], r_ws1[:], r_a1s[:], H)
            dot_heads(cc_row[0:1, H : 2 * H], r_ws1[:], r_a1d[:], H)
            # M = [M1row0(4)|M1row1(4)|M2row0(4)|M2row1(4)]
            m_row = cp.tile([1, 4 * H], F32)
            dot_heads(m_row[0:1, 0:H], r_we1[0:1, 0:HC], r_ae1[:], H)
            dot_heads(m_row[0:1, H : 2 * H], r_we1[0:1, HC : 2 * HC], r_ae1[:], H)
            dot_heads(m_row[0:1, 2 * H : 3 * H], r_we2[0:1, 0:HC], r_ae2[:], H)
            dot_heads(m_row[0:1, 3 * H : 4 * H], r_we2[0:1, HC : 2 * HC], r_ae2[:], H)
            # cds = [cd(3) | cdd(3)]: cd[j] = sum_c wd[j,c]*ads[c]
            cds_row = cp.tile([1, 6], F32)
            for k, arow in ((0, r_ads), (3, r_add)):
                nc.vector.tensor_tensor(
                    out=brd(scratch[:], [scratch[:].ap[0], [C, 3], [1, C]]),
                    in0=brd(r_wdf[:], [r_wdf[:].ap[0], [C, 3], [1, C]]),
                    in1=brd(arow[:], [arow[:].ap[0], [0, 3], [1, C]]),
                    op=OP.mult,
                )
                nc.vector.reduce_sum(
                    out=cds_row[0:1, k : k + 3],
                    in_=brd(scratch[:], [scratch[:].ap[0], [C, 3], [1, C]]),
                    axis=mybir.AxisListType.X,
                )

            def prep(row_ap, width, tag):
                t = cp.tile([P, width], F32, tag=tag)
                nc.gpsimd.partition_broadcast(t[:], row_ap)
                return t

            cc_rep = prep(cc_row[:], 2 * H, "cc_rep")
            m_rep = prep(m_row[:], 4 * H, "m_rep")
            cds_rep = prep(cds_row[:], 6, "cds_rep")
            w1_rep = prep(r_ws1[:], HC, "w1_rep")
            bs1_rep = prep(r_bs1[:], HC, "bs1_rep")
            a2s_rep = prep(r_a2s[:], HC, "a2s_rep")
            a2d_rep = prep(r_a2d[:], HC, "a2d_rep")
            bs2_rep = prep(r_bs2[:], C, "bs2_rep")
            bd_rep = prep(r_bd[:], C, "bd_rep")
            bv1_rep = prep(r_bv1[:], C, "bv1_rep")
            bv2_rep = prep(r_bv2[:], 1, "bv2_rep")

            ws2_sb = cp.tile([P, 2, HC], F32)  # [i_chunk][128, 256]
            nc.sync.dma_start(out=ws2_sb[:, 0, :], in_=ws2[0:P, :])
            nc.sync.dma_start(out=ws2_sb[:, 1, :], in_=ws2[P : 2 * P, :])
            ws2f = cp.tile([P, 2, HC], F32R)
            nc.vector.tensor_copy(out=ws2f[:], in_=ws2_sb[:])
            wd_sb = cp.tile([3, C], F32)
            nc.sync.dma_start(out=wd_sb[:], in_=wd[:])
            wv1_sb = cp.tile([C, C], F32)
            nc.sync.dma_start(out=wv1_sb[:], in_=wv1[:])
            wv2_sb = cp.tile([C, 1], F32)
            nc.sync.dma_start(out=wv2_sb[:], in_=wv2[:])

            # resident metadata
            msrc_sb = mp.tile([P, T], I32)
            nc.sync.dma_start(out=msrc_sb[:], in_=m_src[:])
            mdl_sb = mp.tile([P, T], I32)
            nc.sync.dma_start(out=mdl_sb[:], in_=m_dl[:])
            mrank_sb = mp.tile([P, T], I32)
            nc.sync.dma_start(out=mrank_sb[:], in_=m_rank[:])
            ea_sb = mp.tile([P, T, 2], F32)
            nc.sync.dma_start(out=ea_sb[:], in_=ea_in[:])

            # ---------------- P1: alE pre-pass ----------------------------
            alE = mp.tile([P, T, 2 * H], F32)
            tse = mp.tile([P, T, 1], F32)
            for li in range(2):  # layer 1/2
                for h in range(H):
                    k = li * H + h
                    nc.vector.tensor_scalar(
                        out=alE[:, :, k : k + 1],
                        in0=ea_sb[:, :, 0:1],
                        scalar1=m_rep[:, 2 * li * H + h : 2 * li * H + h + 1],
                        scalar2=None,
                        op0=OP.mult,
                    )
                    nc.vector.tensor_scalar(
                        out=tse[:],
                        in0=ea_sb[:, :, 1:2],
                        scalar1=m_rep[:, (2 * li + 1) * H + h : (2 * li + 1) * H + h + 1],
                        scalar2=None,
                        op0=OP.mult,
                    )
                    nc.vector.tensor_tensor(
                        out=alE[:, :, k : k + 1],
                        in0=alE[:, :, k : k + 1],
                        in1=tse[:],
                        op=OP.add,
                    )

            # ---------------- P2: dstvA table -----------------------------
            for i in range(nwin):
                xo = wp.tile([P, 4], F32, tag="xo")
                nc.sync.dma_start(out=xo[:], in_=x_own[i * P : (i + 1) * P, :])
                dva = wp.tile([P, 5], F32, tag="dva")
                nc.vector.tensor_tensor(
                    out=dva[:, 0:H],
                    in0=cc_rep[:, H : 2 * H],
                    in1=xo[:, 0:1].to_broadcast([P, H]),
                    op=OP.mult,
                )
                t3 = wp.tile([P, 3], F32, tag="t3")
                nc.vector.tensor_tensor(
                    out=t3[:], in0=xo[:, 1:4], in1=cds_rep[:, 3:6], op=OP.mult
                )
                nc.vector.reduce_sum(
                    out=dva[:, 4:5], in_=t3[:], axis=mybir.AxisListType.X
                )
                nc.sync.dma_start(out=dstvA[i * P : (i + 1) * P, :], in_=dva[:])
            zpad = wp.tile([P, 5], F32, tag="dva")
            nc.vector.memset(zpad[:], 0.0)
            nc.sync.dma_start(out=dstvA[R : R + P, :], in_=zpad[:])
            nc.sync.dma_start(out=dstv2[R : R + P, :], in_=zpad[:, 0:4])

            # ---------------- P3: pass A edge loop ------------------------
            rA = mp.tile([P, nwin, 2 * H], F32)  # [r1(4) | rd(3) | pad]
            ti = 0
            for w in range(nwin):
                nt = tpw[w]
                psA = pp.tile([P, 12], F32, tag="win", space="PSUM")
                j0 = 0
                while j0 < nt:
                    kb = min(KB, nt - j0)
                    xg = wp.tile([P, KB, 4], F32, tag="xg")
                    dvg = wp.tile([P, KB, 5], F32, tag="dvg")
                    for j in range(kb):
                        t = ti + j0 + j
                        nc.gpsimd.indirect_dma_start(
                            out=xg[:, j, :],
                            out_offset=None,
                            in_=x_pad[:],
                            in_offset=bass.IndirectOffsetOnAxis(
                                ap=msrc_sb[:, t : t + 1], axis=0
                            ),
                        )
                        nc.gpsimd.indirect_dma_start(
                            out=dvg[:, j, :],
                            out_offset=None,
                            in_=dstvA[:],
                            in_offset=bass.IndirectOffsetOnAxis(
                                ap=mdl_sb[:, t : t + 1], axis=0
                            ),
                        )
                    tt = ti + j0
                    al = wp.tile([P, KB, 5], F32, tag="al")
                    # al1 = x0*c1 + dvA + alE1
                    nc.vector.tensor_tensor(
                        out=al[:, 0:kb, 0:H],
                        in0=brd(cc_rep[:], [cc_rep[:].ap[0], [0, kb], [1, H]]),
                        in1=brd(xg[:], [xg[:].ap[0], [4, kb], [0, H]]),
                        op=OP.mult,
                    )
                    nc.vector.tensor_tensor(
                        out=al[:, 0:kb, 0:H],
                        in0=al[:, 0:kb, 0:H],
                        in1=dvg[:, 0:kb, 0:H],
                        op=OP.add,
                    )
                    nc.vector.tensor_tensor(
                        out=al[:, 0:kb, 0:H],
                        in0=al[:, 0:kb, 0:H],
                        in1=alE[:, tt : tt + kb, 0:H],
                        op=OP.add,
                    )
                    # ald = xd.cd + dvA[4]
                    t3b = wp.tile([P, KB, 3], F32, tag="t3b")
                    nc.vector.tensor_tensor(
                        out=t3b[:, 0:kb, :],
                        in0=xg[:, 0:kb, 1:4],
                        in1=brd(cds_rep[:], [cds_rep[:].ap[0], [0, kb], [1, 3]]),
                        op=OP.mult,
                    )
                    nc.vector.reduce_sum(
                        out=al[:, 0:kb, 4:5],
                        in_=t3b[:, 0:kb, :],
                        axis=mybir.AxisListType.X,
                    )
                    nc.vector.tensor_tensor(
                        out=al[:, 0:kb, 4:5],
                        in0=al[:, 0:kb, 4:5],
                        in1=dvg[:, 0:kb, 4:5],
                        op=OP.add,
                    )
                    # leaky relu (slope 0.2) then exp
                    t5 = wp.tile([P, KB, 5], F32, tag="t5")
                    nc.vector.tensor_scalar(
                        out=t5[:, 0:kb, :], in0=al[:, 0:kb, :],
                        scalar1=0.2, scalar2=None, op0=OP.mult,
                    )
                    nc.vector.tensor_tensor(
                        out=al[:, 0:kb, :], in0=al[:, 0:kb, :], in1=t5[:, 0:kb, :],
                        op=OP.max,
                    )
                    rhs = wp.tile([P, KB, 12], F32R, tag="rhs")
                    nc.scalar.activation(rhs[:, 0:kb, 0:5], al[:, 0:kb, :], AF.Exp)
                    nc.vector.tensor_tensor(
                        out=rhs[:, 0:kb, 5:9],
                        in0=rhs[:, 0:kb, 0:4],
                        in1=brd(xg[:], [xg[:].ap[0], [4, kb], [0, 4]]),
                        op=OP.mult,
                    )
                    nc.vector.tensor_tensor(
                        out=rhs[:, 0:kb, 9:12],
                        in0=xg[:, 0:kb, 1:4],
                        in1=brd(rhs[:], [rhs[:].ap[0], [12, kb], [0, 3]], offset=rhs[:].offset + 4),
                        op=OP.mult,
                    )
                    oh = wp.tile([P, KB, P], F32R, tag="oh")
                    nc.vector.tensor_tensor(
                        out=oh[:, 0:kb, :],
                        in0=mrank_sb[:, tt : tt + kb].to_broadcast([P, kb, P]),
                        in1=brd(iota_mat[:], [iota_mat[:].ap[0], [0, kb], [1, P]]),
                        op=OP.is_equal,
                    )
                    for j in range(kb):
                        nc.tensor.matmul(
                            out=psA[:],
                            lhsT=oh[:, j, :],
                            rhs=rhs[:, j, :],
                            start=(j0 + j == 0),
                            stop=(j0 + j == nt - 1),
                        )
                    j0 += kb
                # window epilogue A: r1 = t1/den1, rd = td/dend
                den = wp.tile([P, 5], F32, tag="den")
                nc.vector.tensor_scalar(
                    out=den[:], in0=psA[:, 0:5], scalar1=EPS, scalar2=None, op0=OP.add
                )
                nc.vector.reciprocal(out=den[:], in_=den[:])
                nc.vector.tensor_tensor(
                    out=rA[:, w, 0:4], in0=psA[:, 5:9], in1=den[:, 0:4], op=OP.mult
                )
                nc.vector.tensor_tensor(
                    out=rA[:, w, 4:7],
                    in0=psA[:, 9:12],
                    in1=den[:, 4:5].to_broadcast([P, 3]),
                    op=OP.mult,
                )
                ti += nt

            # ---------------- P4: node phase ------------------------------
            hd_sb = mp.tile([P, nwin, C], F32)
            for i in range(nwin):
                h1 = wp.tile([P, HC], F32, tag="h1")
                nc.vector.tensor_tensor(
                    out=brd(h1[:], [h1[:].ap[0], [C, H], [1, C]]),
                    in0=brd(w1_rep[:], [w1_rep[:].ap[0], [C, H], [1, C]]),
                    in1=brd(rA[:], [rA[:].ap[0], [1, H], [0, C]],
                            offset=rA[:].offset + i * 2 * H),
                    op=OP.mult,
                )
                nc.vector.tensor_tensor(out=h1[:], in0=h1[:], in1=bs1_rep[:], op=OP.add)
                # elu
                e1 = wp.tile([P, HC], F32, tag="e1")
                nc.vector.tensor_scalar(
                    out=e1[:], in0=h1[:], scalar1=0.0, scalar2=None, op0=OP.min
                )
                nc.scalar.activation(e1[:], e1[:], AF.Exp)
                nc.vector.tensor_scalar(
                    out=e1[:], in0=e1[:], scalar1=-1.0, scalar2=None, op0=OP.add
                )
                nc.vector.tensor_scalar(
                    out=h1[:], in0=h1[:], scalar1=0.0, scalar2=None, op0=OP.max
                )
                nc.vector.tensor_tensor(out=h1[:], in0=h1[:], in1=e1[:], op=OP.add)
                # transpose h1e chunks
                h1t = wp.tile([P, 2, P], F32R, tag="h1t")
                for ch in range(2):
                    pst = pt.tile([P, P], F32, tag="tr", space="PSUM")
                    nc.tensor.transpose(
                        out=pst[:], in_=h1[:, ch * P : (ch + 1) * P], identity=ident[:]
                    )
                    nc.vector.tensor_copy(out=h1t[:, ch, :], in_=pst[:])
                ph2 = pt.tile([P, HC], F32, tag="mm", space="PSUM")
                for ch in range(2):
                    nc.tensor.matmul(
                        out=ph2[:],
                        lhsT=h1t[:, ch, :],
                        rhs=ws2f[:, ch, :],
                        start=(ch == 0),
                        stop=(ch == 1),
                    )
                # H2 row = [h2 | s2src]; also s2dst -> dstv2
                h2row = wp.tile([P, HR], F32, tag="h2row")
                nc.vector.tensor_copy(out=h2row[:, 0:HC], in_=ph2[:])
                tm = wp.tile([P, HC], F32, tag="tm")
                nc.vector.tensor_tensor(out=tm[:], in0=ph2[:], in1=a2s_rep[:], op=OP.mult)
                nc.vector.reduce_sum(
                    out=h2row[:, HC : HC + H],
                    in_=brd(tm[:], [tm[:].ap[0], [C, H], [1, C]]),
                    axis=mybir.AxisListType.X,
                )
                nc.vector.tensor_tensor(out=tm[:], in0=ph2[:], in1=a2d_rep[:], op=OP.mult)
                sd2 = wp.tile([P, H], F32, tag="sd2")
                nc.vector.reduce_sum(
                    out=sd2[:],
                    in_=brd(tm[:], [tm[:].ap[0], [C, H], [1, C]]),
                    axis=mybir.AxisListType.X,
                )
                nc.sync.dma_start(out=dstv2[i * P : (i + 1) * P, :], in_=sd2[:])
                nc.sync.dma_start(out=h2slice[i * P : (i + 1) * P, :], in_=h2row[:])
                # dynamic head output for own nodes
                prd = pt.tile([P, P], F32, tag="tr", space="PSUM")
                nc.tensor.transpose(out=prd[0:3, :], in_=rA[:, i, 4:7], identity=ident[:])
                rdt = wp.tile([3, P], F32, tag="rdt")
                nc.vector.tensor_copy(out=rdt[:], in_=prd[0:3, :])
                phd = pt.tile([P, C], F32, tag="mm", space="PSUM")
                nc.tensor.matmul(
                    out=phd[:], lhsT=rdt[:], rhs=wd_sb[:], start=True, stop=True
                )
                nc.vector.tensor_tensor(
                    out=hd_sb[:, i, :], in0=phd[:], in1=bd_rep[:], op=OP.add
                )

            # ---------------- P5: allgather -------------------------------
            nc.gpsimd.collective_compute(
                "AllGather",
                OP.bypass,
                replica_groups=[list(range(W))],
                ins=[h2slice[:]],
                outs=[H2ext[:]],
            )

            # ---------------- P6: pass B edge loop ------------------------
            h_sb = mp.tile([P, nwin, C], F32)
            ti = 0
            for w in range(nwin):
                nt = tpw[w]
                psB = pp.tile([P, HR], F32, tag="win", space="PSUM")
                j0 = 0
                while j0 < nt:
                    kb = min(KB, nt - j0)
                    hg = wp.tile([P, KB, HR], F32, tag="hg")
                    dv2 = wp.tile([P, KB, 4], F32, tag="dv2")
                    for j in range(kb):
                        t = ti + j0 + j
                        nc.gpsimd.indirect_dma_start(
                            out=hg[:, j, :],
                            out_offset=None,
                            in_=H2ext[:],
                            in_offset=bass.IndirectOffsetOnAxis(
                                ap=msrc_sb[:, t : t + 1], axis=0
                            ),
                        )
                        nc.gpsimd.indirect_dma_start(
                            out=dv2[:, j, :],
                            out_offset=None,
                            in_=dstv2[:],
                            in_offset=bass.IndirectOffsetOnAxis(
                                ap=mdl_sb[:, t : t + 1], axis=0
                            ),
                        )
                    tt = ti + j0
                    al2 = wp.tile([P, KB, H], F32, tag="al2")
                    nc.vector.tensor_tensor(
                        out=al2[:, 0:kb, :],
                        in0=hg[:, 0:kb, HC : HC + H],
                        in1=dv2[:, 0:kb, :],
                        op=OP.add,
                    )
                    nc.vector.tensor_tensor(
                        out=al2[:, 0:kb, :],
                        in0=al2[:, 0:kb, :],
                        in1=alE[:, tt : tt + kb, H : 2 * H],
                        op=OP.add,
                    )
                    t4 = wp.tile([P, KB, H], F32, tag="t4")
                    nc.vector.tensor_scalar(
                        out=t4[:, 0:kb, :], in0=al2[:, 0:kb, :],
                        scalar1=0.2, scalar2=None, op0=OP.mult,
                    )
                    nc.vector.tensor_tensor(
                        out=al2[:, 0:kb, :], in0=al2[:, 0:kb, :], in1=t4[:, 0:kb, :],
                        op=OP.max,
                    )
                    rhsB = wp.tile([P, KB, HR], F32R, tag="rhsB")
                    nc.scalar.activation(
                        rhsB[:, 0:kb, HC : HC + H], al2[:, 0:kb, :], AF.Exp
                    )
                    # msgs = hg * ex (per-head broadcast over 64 channels)
                    nc.vector.tensor_tensor(
                        out=brd(rhsB[:], [rhsB[:].ap[0], [HR, kb], [C, H], [1, C]]),
                        in0=brd(hg[:], [hg[:].ap[0], [HR, kb], [C, H], [1, C]]),
                        in1=brd(rhsB[:], [rhsB[:].ap[0], [HR, kb], [1, H], [0, C]],
                                offset=rhsB[:].offset + HC),
                        op=OP.mult,
                    )
                    ohB = wp.tile([P, KB, P], F32R, tag="ohB")
                    nc.vector.tensor_tensor(
                        out=ohB[:, 0:kb, :],
                        in0=mrank_sb[:, tt : tt + kb].to_broadcast([P, kb, P]),
                        in1=brd(iota_mat[:], [iota_mat[:].ap[0], [0, kb], [1, P]]),
                        op=OP.is_equal,
                    )
                    for j in range(kb):
                        nc.tensor.matmul(
                            out=psB[:],
                            lhsT=ohB[:, j, :],
                            rhs=rhsB[:, j, :],
                            start=(j0 + j == 0),
                            stop=(j0 + j == nt - 1),
                        )
                    j0 += kb
                # window epilogue B
                dn2 = wp.tile([P, H], F32, tag="dn2")
                nc.vector.tensor_scalar(
                    out=dn2[:], in0=psB[:, HC : HC + H], scalar1=EPS, scalar2=None,
                    op0=OP.add,
                )
                nc.vector.reciprocal(out=dn2[:], in_=dn2[:])
                agg = wp.tile([P, HC], F32, tag="agg")
                nc.vector.tensor_tensor(
                    out=brd(agg[:], [agg[:].ap[0], [C, H], [1, C]]),
                    in0=brd(psB[:], [psB[:].ap[0], [C, H], [1, C]]),
                    in1=brd(dn2[:], [dn2[:].ap[0], [1, H], [0, C]]),
                    op=OP.mult,
                )
                # mean over heads (stride trick: inner dim = heads)
                hf = wp.tile([P, C], F32, tag="hf")
                nc.vector.reduce_sum(
                    out=hf[:],
                    in_=brd(agg[:], [agg[:].ap[0], [1, C], [C, H]]),
                    axis=mybir.AxisListType.X,
                )
                nc.vector.tensor_scalar(
                    out=hf[:], in0=hf[:], scalar1=0.25, scalar2=None, op0=OP.mult
                )
                nc.vector.tensor_tensor(out=hf[:], in0=hf[:], in1=bs2_rep[:], op=OP.add)
                nc.vector.tensor_tensor(
                    out=h_sb[:, w, :], in0=hf[:], in1=hd_sb[:, w, :], op=OP.add
                )
                ti += nt

            # ---------------- P7: pooling + value head --------------------
            pg = pp.tile([gpc, C], F32, tag="win", space="PSUM")
            for w in range(nwin):
                pm = wp.tile([P, gpc], F32, tag="pm")
                nc.sync.dma_start(out=pm[:], in_=pmask[w * P : (w + 1) * P, :])
                nc.tensor.matmul(
                    out=pg[:],
                    lhsT=pm[:],
                    rhs=h_sb[:, w, :],
                    start=(w == 0),
                    stop=(w == nwin - 1),
                )
            g_sb = wp.tile([gpc, C], F32, tag="g_sb")
            nc.vector.tensor_copy(out=g_sb[:], in_=pg[:])
            pgt = pt.tile([C, gpc], F32, tag="tr", space="PSUM")
            nc.tensor.transpose(
                out=pgt[:], in_=g_sb[:], identity=ident[0:gpc, 0:gpc]
            )
            gt_sb = wp.tile([C, gpc], F32, tag="gt_sb")
            nc.vector.tensor_copy(out=gt_sb[:], in_=pgt[:])
            pv1 = pt.tile([gpc, C], F32, tag="mm", space="PSUM")
            nc.tensor.matmul(out=pv1[:], lhsT=gt_sb[:], rhs=wv1_sb[:], start=True, stop=True)
            a_sb = wp.tile([gpc, C], F32, tag="a_sb")
            nc.vector.tensor_tensor(
                out=a_sb[:], in0=pv1[:], in1=bv1_rep[0:gpc, :], op=OP.add
            )
            nc.vector.tensor_scalar(
                out=a_sb[:], in0=a_sb[:], scalar1=0.0, scalar2=None, op0=OP.max
            )
            pat = pt.tile([C, gpc], F32, tag="tr", space="PSUM")
            nc.tensor.transpose(out=pat[:], in_=a_sb[:], identity=ident[0:gpc, 0:gpc])
            at_sb = wp.tile([C, gpc], F32, tag="at_sb")
            nc.vector.tensor_copy(out=at_sb[:], in_=pat[:])
            pv2 = pt.tile([gpc, 1], F32, tag="mm", space="PSUM")
            nc.tensor.matmul(out=pv2[:], lhsT=at_sb[:], rhs=wv2_sb[:], start=True, stop=True)
            vres = wp.tile([gpc, 1], F32, tag="vres")
            nc.vector.tensor_tensor(
                out=vres[:], in0=pv2[:], in1=bv2_rep[0:gpc, :], op=OP.add
            )
            nc.sync.dma_start(out=v_out[:], in_=vres[:])

    nc.compile()
    return nc


# ----------------------------------------------------------------------------
# in_maps assembly
# ----------------------------------------------------------------------------

def make_in_maps(plan, x_pad, per_core_arrays, weights):
    w = {k: np.ascontiguousarray(v, np.float32) for k, v in weights.items()}
    shared = dict(
        x_pad=x_pad,
        ws1=w["Ws1"].reshape(1, HC),
        a1s=w["as_src1"].reshape(1, HC),
        a1d=w["as_dst1"].reshape(1, HC),
        we1=w["We1"].reshape(1, 2 * HC),
        ae1=w["ae1"].reshape(1, HC),
        bs1=w["bs1"].reshape(1, HC),
        ws2=w["Ws2"],
        a2s=w["as_src2"].reshape(1, HC),
        a2d=w["as_dst2"].reshape(1, HC),
        we2=w["We2"].reshape(1, 2 * HC),
        ae2=w["ae2"].reshape(1, HC),
        bs2=w["bs2"].reshape(1, C),
        wd=w["Wd"],
        wdf=w["Wd"].reshape(1, 3 * C),
        ads=w["ad_src"].reshape(1, C),
        add=w["ad_dst"].reshape(1, C),
        bd=w["bd"].reshape(1, C),
        wv1=w["Wv1"],
        bv1=w["bv1"].reshape(1, C),
        wv2=w["Wv2"],
        bv2=w["bv2"].reshape(1, 1),
    )
    in_maps = []
    for c in range(plan.W):
        m = dict(shared)
        m.update(per_core_arrays[c])
        in_maps.append(m)
    return in_maps


_CACHE = {}


def kernel(**inputs):
    x = np.asarray(inputs["x"])
    edge_attr = np.asarray(inputs["edge_attr"])
    edge_index = np.asarray(inputs["edge_index"])
    batch = np.asarray(inputs["batch"])
    G = 64
    W = 8

    plan, x_pad, pca = host_prep(x, edge_attr, edge_index, batch, G, W)
    key = (plan.R, plan.T, tuple(plan.tiles_per_window))
    if key not in _CACHE:
        _CACHE[key] = build_bass(plan)
    nc = _CACHE[key]
    weights = {k: inputs[k] for k in (
        "Ws1", "as_src1", "as_dst1", "We1", "ae1", "bs1",
        "Ws2", "as_src2", "as_dst2", "We2", "ae2", "bs2",
        "Wd", "ad_src", "ad_dst", "bd", "Wv1", "bv1", "Wv2", "bv2")}
    in_maps = make_in_maps(plan, x_pad, pca, weights)
    from concourse.bass_utils import run_bass_kernel_spmd
    res = run_bass_kernel_spmd(nc, in_maps, list(range(W)))
    v = np.concatenate([res.results[c]["v"][:, 0] for c in range(W)])
    return v.astype(np.float32)



# revision 6
# speedup vs baseline: 2.6253x; 2.6253x over previous
"""Trainium2 Bass kernel for nn_CriticNetwork (3x GATConv + pool + MLP head).

v2 — SWDGE-call-count optimized 8-way graph/data parallel design.

Key structure (per core, graphs are contiguous node ranges since batch is
sorted; core c owns graphs [8c, 8c+8) = a node range, and all edges whose
dst lands in it):
- Edges sorted by (dst window, src half, dst) and chopped into 128-edge
  tiles that never cross a 128-node dst window nor a src half (src halves:
  cores 0-3 / 4-7, so H2 gather indices fit int16 for dma_gather).
- Host ships pure index/permutation data per edge lane: x[src]/x[dst]
  rows (a gather of *input* tensors = sharding), edge_attr rows, rank
  one-hot matrices (oh for scatter-accumulate matmuls, ohT for dst-value
  broadcast matmuls), and int16 gather index tables. All weight math runs
  on device.
- Pass A (layer-1 + dynamic-layer attention, rank-1 in x): per-edge logits
  built by DVE from resident x-row tables, Prelu+Exp on the Act engine,
  per-window scatter matmul (oh^T @ msgs) into PSUM. Node phase fused per
  window: h1 -> elu -> h2 = h1 @ [Ws2 | Ws2@A2s | Ws2@A2d] (attention dot
  columns fused into the same matmul), h2 row written to h2slice.
- One AllGather of the padded h2 rows (bf16) -> H2ext.
- Pass B (layer-2): per-window dma_gather (2 calls: src halves) of 768B
  h2 rows, dst attention values broadcast via ohT matmul, msgs = hg * ex,
  scatter matmul, softmax epilogue, + dynamic head -> pooled by a
  host-built (1/count) mask matmul; tiny value-head MLP per core.
"""

import numpy as np
import ml_dtypes

import concourse.bacc as bacc
import concourse.bass as bass
import concourse.mybir as mybir
import concourse.tile as tile
from concourse.masks import make_identity

F32 = mybir.dt.float32
BF16 = mybir.dt.bfloat16
I16 = mybir.dt.int16
AF = mybir.ActivationFunctionType
OP = mybir.AluOpType
NPBF = ml_dtypes.bfloat16

P = 128
H = 4          # heads (static encoder)
C = 64         # channels per head
HC = H * C     # 256
HR = 384       # padded H2 row width (bf16) -> 768B, multiple of 256B
EPS = 1e-16
NEG = 0.2


def brd(ap, pattern, offset=None):
    """Manual broadcast: new AP over same tensor with given [step, count] list."""
    return bass.AP(ap.tensor, ap.offset if offset is None else offset, pattern)


# ----------------------------------------------------------------------------
# Host-side sharding / planning (pure index & layout work; no weight math)
# ----------------------------------------------------------------------------

class Plan:
    pass


def host_prep(x, edge_attr, edge_index, batch, n_graphs, n_cores):
    N = x.shape[0]
    W = n_cores
    gpc = n_graphs // W
    assert gpc * W == n_graphs

    batch = np.asarray(batch).astype(np.int64)
    src = np.asarray(edge_index[0]).astype(np.int64)
    dst = np.asarray(edge_index[1]).astype(np.int64)
    ea = np.asarray(edge_attr).astype(np.float32)
    x = np.asarray(x).astype(np.float32)

    node_start = np.searchsorted(batch, np.arange(n_graphs + 1))
    core_ns = node_start[0::gpc]            # [W+1]
    nk = np.diff(core_ns)
    R = int(128 * np.ceil(nk.max() / 128))
    nwin = R // 128
    NP = W * R
    HALF = (W // 2) * R

    core_of = np.searchsorted(core_ns, np.arange(N), side="right") - 1
    pid = core_of * R + (np.arange(N) - core_ns[core_of])

    counts = np.bincount(batch, minlength=n_graphs).astype(np.float32)
    assert (counts > 0).all()

    dcore = np.searchsorted(core_ns, dst, side="right") - 1
    src_half = (pid[src] >= HALF).astype(np.int64)

    # per-core edge lists sorted by (window, src half, dst)
    per_core_sorted = []
    run_len = np.zeros((W, nwin, 2), np.int64)
    for c in range(W):
        m = dcore == c
        dl = (dst[m] - core_ns[c]).astype(np.int64)
        sh = src_half[m]
        w = dl >> 7
        order = np.lexsort((dl, sh, w))
        e_spid = pid[src[m]][order]
        e_dl = dl[order]
        e_w = w[order]
        e_sh = sh[order]
        e_ea = ea[m][order]
        e_xs = x[src[m]][order]
        e_xd = x[dst[m]][order]
        for wi in range(nwin):
            for h in range(2):
                run_len[c, wi, h] = int(np.sum((e_w == wi) & (e_sh == h)))
        per_core_sorted.append((e_spid, e_dl, e_ea, e_xs, e_xd))

    # uniform tile structure across cores: per (window, half) tile count
    tpw = np.maximum(1, np.ceil(run_len.max(axis=0) / P).astype(np.int64))  # [nwin,2]
    T = int(tpw.sum())

    tile_w = []          # window of each tile
    tile_h = []          # half of each tile
    for wi in range(nwin):
        for h in range(2):
            for _ in range(int(tpw[wi, h])):
                tile_w.append(wi)
                tile_h.append(h)

    per_core_arrays = []
    for c in range(W):
        e_spid, e_dl, e_ea, e_xs, e_xd = per_core_sorted[c]
        xed = np.zeros((T, P, 8), np.float32)
        eat = np.zeros((T, P, 2), np.float32)
        ohb = np.zeros((T, P, 256), NPBF)
        idx16 = np.zeros((T * P,), np.int16)
        e0 = 0
        ti = 0
        for wi in range(nwin):
            for h in range(2):
                n_run = int(run_len[c, wi, h])
                for j in range(int(tpw[wi, h])):
                    a = e0 + P * j
                    b = min(a + P, e0 + n_run)
                    if b > a:
                        nn = b - a
                        xed[ti, :nn, 0:4] = e_xs[a:b]
                        xed[ti, :nn, 4:8] = e_xd[a:b]
                        eat[ti, :nn] = e_ea[a:b]
                        rank = (e_dl[a:b] - P * wi).astype(np.int64)
                        lanes = np.arange(nn)
                        ohb[ti, lanes, rank] = 1.0
                        ohb[ti, rank, 128 + lanes] = 1.0
                        loc = e_spid[a:b] - np.where(e_spid[a:b] >= HALF, HALF, 0)
                        idx16[ti * P:ti * P + nn] = loc.astype(np.int16)
                    ti += 1
                e0 += n_run
        assert ti == T and e0 == len(e_dl)

        # wrap idx into 16-partition layout, replicate across 8 groups
        idx_w = idx16.reshape(T * P // 16, 16).T          # [16, T*8]
        idx_rep = np.tile(idx_w, (8, 1))                  # [128, T*8]

        ns, ne = int(core_ns[c]), int(core_ns[c + 1])
        pmask = np.zeros((R, gpc), np.float32)
        gidx = (batch[ns:ne] - c * gpc).astype(np.int64)
        pmask[np.arange(ne - ns), gidx] = 1.0 / counts[batch[ns:ne]]

        per_core_arrays.append(dict(
            xed=np.ascontiguousarray(xed.transpose(1, 0, 2)).astype(NPBF),
            ea_t=np.ascontiguousarray(eat.transpose(1, 0, 2)),
            ohb=np.ascontiguousarray(ohb.transpose(1, 0, 2)),
            idx16=np.ascontiguousarray(idx_rep),
            pmask=pmask.astype(NPBF),
        ))

    plan = Plan()
    plan.W = W
    plan.R = R
    plan.NP = NP
    plan.HALF = HALF
    plan.nwin = nwin
    plan.T = T
    plan.tpw = tpw                      # [nwin, 2]
    plan.gpc = gpc
    return plan, per_core_arrays


# ----------------------------------------------------------------------------
# Device program
# ----------------------------------------------------------------------------

def build_bass(plan):
    W, R, NP, nwin, T = plan.W, plan.R, plan.NP, plan.nwin, plan.T
    tpw = plan.tpw
    gpc = plan.gpc
    TMAX = int(tpw.sum(axis=1).max())   # max tiles in any window

    nc = bacc.Bacc("TRN2", target_bir_lowering=False, debug=False, num_devices=W)

    def dp(name, shape, dtype=F32, out=False):
        return nc.declare_dram_parameter(name, list(shape), dtype, isOutput=out)

    xed_in = dp("xed", [P, T, 8], BF16)
    ea_in = dp("ea_t", [P, T, 2])
    ohb_in = dp("ohb", [P, T, 256], BF16)
    idx_in = dp("idx16", [P, T * 8], I16)
    pmask = dp("pmask", [R, gpc], BF16)

    ws1 = dp("ws1", [1, HC])
    a1s = dp("a1s", [1, HC])
    a1d = dp("a1d", [1, HC])
    we1 = dp("we1", [1, 2 * HC])
    ae1 = dp("ae1", [1, HC])
    bs1 = dp("bs1", [1, HC])
    ws2 = dp("ws2", [HC, HC])
    a2s = dp("a2s", [1, HC])
    a2d = dp("a2d", [1, HC])
    we2 = dp("we2", [1, 2 * HC])
    ae2 = dp("ae2", [1, HC])
    bs2 = dp("bs2", [1, C])
    wd = dp("wd", [3, C])
    wdf = dp("wdf", [1, 3 * C])
    ads = dp("ads", [1, C])
    add_ = dp("add", [1, C])
    bd = dp("bd", [1, C])
    wv1 = dp("wv1", [C, C])
    bv1 = dp("bv1", [1, C])
    wv2 = dp("wv2", [C, 1])
    bv2 = dp("bv2", [1, 1])

    v_out = dp("v", [gpc, 1], out=True)

    h2slice = nc.dram_tensor("h2slice", [R, HR], BF16)
    if W > 4:
        H2ext = nc.dram_tensor("H2ext", [NP, HR], BF16, addr_space="Shared")
    else:
        H2ext = nc.dram_tensor("H2ext", [NP, HR], BF16)

    with tile.TileContext(nc) as tc:
        with (
            tc.tile_pool(name="const", bufs=1) as cp,
            tc.tile_pool(name="meta", bufs=1) as mp,
        ):
            # ---------------- constants ---------------------------------
            ident = cp.tile([P, P], F32)
            make_identity(nc, ident[:])
            ident_bf = cp.tile([P, P], BF16)
            nc.vector.tensor_copy(out=ident_bf[:], in_=ident[:])

            def load_row(dram, width, tag):
                t = cp.tile([1, width], F32, tag=tag)
                nc.sync.dma_start(out=t[:], in_=dram[0:1, 0:width])
                return t

            r_ws1 = load_row(ws1, HC, "r_ws1")
            r_a1s = load_row(a1s, HC, "r_a1s")
            r_a1d = load_row(a1d, HC, "r_a1d")
            r_we1 = load_row(we1, 2 * HC, "r_we1")
            r_ae1 = load_row(ae1, HC, "r_ae1")
            r_bs1 = load_row(bs1, HC, "r_bs1")
            r_a2s = load_row(a2s, HC, "r_a2s")
            r_a2d = load_row(a2d, HC, "r_a2d")
            r_we2 = load_row(we2, 2 * HC, "r_we2")
            r_ae2 = load_row(ae2, HC, "r_ae2")
            r_bs2 = load_row(bs2, C, "r_bs2")
            r_wdf = load_row(wdf, 3 * C, "r_wdf")
            r_ads = load_row(ads, C, "r_ads")
            r_add = load_row(add_, C, "r_add")
            r_bd = load_row(bd, C, "r_bd")
            r_bv1 = load_row(bv1, C, "r_bv1")
            r_bv2 = load_row(bv2, 1, "r_bv2")

            scratch = cp.tile([1, 2 * HC], F32)

            def dot_heads(out_ap, wrow, arow, nh):
                nc.vector.tensor_tensor(
                    out=scratch[0:1, 0:nh * C], in0=wrow, in1=arow, op=OP.mult)
                nc.vector.reduce_sum(
                    out=out_ap,
                    in_=brd(scratch[:], [scratch[:].ap[0], [C, nh], [1, C]]),
                    axis=mybir.AxisListType.X)

            # cc = [c1_src(4) | c1_dst(4)]
            cc_row = cp.tile([1, 2 * H], F32)
            dot_heads(cc_row[0:1, 0:H], r_ws1[:], r_a1s[:], H)
            dot_heads(cc_row[0:1, H:2 * H], r_ws1[:], r_a1d[:], H)
            # M = [M1row0(4)|M1row1(4)|M2row0(4)|M2row1(4)]
            m_row = cp.tile([1, 4 * H], F32)
            dot_heads(m_row[0:1, 0:H], r_we1[0:1, 0:HC], r_ae1[:], H)
            dot_heads(m_row[0:1, H:2 * H], r_we1[0:1, HC:2 * HC], r_ae1[:], H)
            dot_heads(m_row[0:1, 2 * H:3 * H], r_we2[0:1, 0:HC], r_ae2[:], H)
            dot_heads(m_row[0:1, 3 * H:4 * H], r_we2[0:1, HC:2 * HC], r_ae2[:], H)
            # cds = [cd_src(3) | cd_dst(3)]
            cds_row = cp.tile([1, 6], F32)
            for k, arow in ((0, r_ads), (3, r_add)):
                nc.vector.tensor_tensor(
                    out=brd(scratch[:], [scratch[:].ap[0], [C, 3], [1, C]]),
                    in0=brd(r_wdf[:], [r_wdf[:].ap[0], [C, 3], [1, C]]),
                    in1=brd(arow[:], [arow[:].ap[0], [0, 3], [1, C]]),
                    op=OP.mult)
                nc.vector.reduce_sum(
                    out=cds_row[0:1, k:k + 3],
                    in_=brd(scratch[:], [scratch[:].ap[0], [C, 3], [1, C]]),
                    axis=mybir.AxisListType.X)

            def prep(row_ap, width, tag):
                t = cp.tile([P, width], F32, tag=tag)
                nc.gpsimd.partition_broadcast(t[:], row_ap)
                return t

            cc_rep = prep(cc_row[:], 2 * H, "cc_rep")
            m_rep = prep(m_row[:], 4 * H, "m_rep")
            cds_rep = prep(cds_row[:], 6, "cds_rep")
            w1_rep = prep(r_ws1[:], HC, "w1_rep")
            bs1_rep = prep(r_bs1[:], HC, "bs1_rep")
            a2s_rep = prep(r_a2s[:], HC, "a2s_rep")
            a2d_rep = prep(r_a2d[:], HC, "a2d_rep")
            bs2_rep = prep(r_bs2[:], C, "bs2_rep")
            bd_rep = prep(r_bd[:], C, "bd_rep")
            bv1_rep = prep(r_bv1[:], C, "bv1_rep")
            bv2_rep = prep(r_bv2[:], 1, "bv2_rep")

            # ws2a = [Ws2 | Ws2@A2s | Ws2@A2d] rows (bf16), per 128-row chunk
            ws2_sb = cp.tile([P, 2, HC], F32)
            nc.sync.dma_start(out=ws2_sb[:, 0, :], in_=ws2[0:P, :])
            nc.sync.dma_start(out=ws2_sb[:, 1, :], in_=ws2[P:2 * P, :])
            ws2a_sb = cp.tile([P, 2, HC + 2 * H], BF16)
            nc.vector.tensor_copy(out=ws2a_sb[:, :, 0:HC], in_=ws2_sb[:])
            tmw = cp.tile([P, HC], F32)
            tmr = cp.tile([P, H], F32)
            for ch in range(2):
                for k, arep in ((0, a2s_rep), (H, a2d_rep)):
                    nc.vector.tensor_tensor(
                        out=tmw[:], in0=ws2_sb[:, ch, :], in1=arep[:], op=OP.mult)
                    nc.vector.reduce_sum(
                        out=tmr[:],
                        in_=brd(tmw[:], [tmw[:].ap[0], [C, H], [1, C]]),
                        axis=mybir.AxisListType.X)
                    nc.vector.tensor_copy(
                        out=ws2a_sb[:, ch, HC + k:HC + k + H], in_=tmr[:])

            wd_sb = cp.tile([3, C], BF16)
            wdt = cp.tile([3, C], F32)
            nc.sync.dma_start(out=wdt[:], in_=wd[:])
            nc.vector.tensor_copy(out=wd_sb[:], in_=wdt[:])
            wv1_sb = cp.tile([C, C], F32)
            nc.sync.dma_start(out=wv1_sb[:], in_=wv1[:])
            wv2_sb = cp.tile([C, 1], F32)
            nc.sync.dma_start(out=wv2_sb[:], in_=wv2[:])

            # ---------------- resident per-edge tables -------------------
            xed_sb = mp.tile([P, T, 8], BF16)
            nc.sync.dma_start(out=xed_sb[:], in_=xed_in[:])
            ea_sb = mp.tile([P, T, 2], F32)
            nc.sync.dma_start(out=ea_sb[:], in_=ea_in[:])
            idx_sb = mp.tile([P, T * 8], I16)
            nc.sync.dma_start(out=idx_sb[:], in_=idx_in[:])
            pm_all = mp.tile([P, nwin, gpc], BF16)
            nc.sync.dma_start(
                out=pm_all[:],
                in_=brd(pmask[:], [[gpc, P], [P * gpc, nwin], [1, gpc]]))

            # alE [P, T, 8] bf16: layer-1 heads 0:4, layer-2 heads 4:8
            alE = mp.tile([P, T, 8], BF16)
            tse = mp.tile([P, T], F32)
            for li in range(2):
                for h in range(H):
                    k = li * H + h
                    nc.vector.tensor_scalar(
                        out=tse[:],
                        in0=ea_sb[:, :, 1],
                        scalar1=m_rep[:, (2 * li + 1) * H + h:(2 * li + 1) * H + h + 1],
                        scalar2=None, op0=OP.mult)
                    nc.vector.scalar_tensor_tensor(
                        out=alE[:, :, k],
                        in0=ea_sb[:, :, 0],
                        scalar=m_rep[:, 2 * li * H + h:2 * li * H + h + 1],
                        in1=tse[:], op0=OP.mult, op1=OP.add)

            rA = mp.tile([P, nwin, 2 * H], F32)      # [r1(4) | rd(3) | pad]
            sd2_all = mp.tile([P, nwin, H], BF16)    # layer-2 dst attn values
            hd_sb = mp.tile([P, nwin, C], F32)       # dynamic head output
            h_sb = mp.tile([P, nwin, C], BF16)       # final node features

            # ---------------- pass A + node phase ------------------------
            with (
                tc.tile_pool(name="ohA", bufs=3) as ohp,
                tc.tile_pool(name="wkA", bufs=3) as wp,
                tc.tile_pool(name="nodeA", bufs=2) as npl,
                tc.tile_pool(name="psA", bufs=2, space="PSUM") as ppa,
                tc.tile_pool(name="psT", bufs=1, space="PSUM") as ppt,
                tc.tile_pool(name="psM", bufs=1, space="PSUM") as ppm,
            ):
                t0 = 0
                for w in range(nwin):
                    nt = int(tpw[w, 0] + tpw[w, 1])
                    ohb_w = ohp.tile([P, TMAX, 256], BF16, tag="ohb")
                    nc.sync.dma_start(
                        out=ohb_w[:, 0:nt, :], in_=ohb_in[:, t0:t0 + nt, :])

                    al = wp.tile([P, TMAX, 5], F32, tag="al")
                    tm4 = wp.tile([P, TMAX, 4], F32, tag="tm4")
                    tm3 = wp.tile([P, TMAX, 3], F32, tag="tm3")
                    xs = xed_sb[:, t0:t0 + nt, :]
                    # al[h] = cc_s[h]*x0s + cc_d[h]*x0d + alE1 ; al[4] = xd_s.cd_s + xd_d.cd_d
                    nc.vector.tensor_tensor(
                        out=al[:, 0:nt, 0:4],
                        in0=brd(cc_rep[:], [cc_rep[:].ap[0], [0, nt], [1, H]]),
                        in1=brd(xs, [xs.ap[0], [8, nt], [0, H]]),
                        op=OP.mult)
                    nc.vector.tensor_tensor(
                        out=tm4[:, 0:nt, :],
                        in0=brd(cc_rep[:], [cc_rep[:].ap[0], [0, nt], [1, H]],
                                offset=cc_rep[:].offset + H),
                        in1=brd(xs, [xs.ap[0], [8, nt], [0, H]],
                                offset=xs.offset + 4),
                        op=OP.mult)
                    nc.vector.tensor_tensor(
                        out=al[:, 0:nt, 0:4], in0=al[:, 0:nt, 0:4],
                        in1=tm4[:, 0:nt, :], op=OP.add)
                    nc.vector.tensor_tensor(
                        out=al[:, 0:nt, 0:4], in0=al[:, 0:nt, 0:4],
                        in1=alE[:, t0:t0 + nt, 0:4], op=OP.add)
                    # dynamic: src + dst dots
                    nc.vector.tensor_tensor(
                        out=tm3[:, 0:nt, :],
                        in0=brd(xs, [xs.ap[0], [8, nt], [1, 3]], offset=xs.offset + 1),
                        in1=brd(cds_rep[:], [cds_rep[:].ap[0], [0, nt], [1, 3]]),
                        op=OP.mult)
                    nc.vector.reduce_sum(
                        out=al[:, 0:nt, 4:5], in_=tm3[:, 0:nt, :],
                        axis=mybir.AxisListType.X)
                    nc.vector.tensor_tensor(
                        out=tm3[:, 0:nt, :],
                        in0=brd(xs, [xs.ap[0], [8, nt], [1, 3]], offset=xs.offset + 5),
                        in1=brd(cds_rep[:], [cds_rep[:].ap[0], [0, nt], [1, 3]],
                                offset=cds_rep[:].offset + 3),
                        op=OP.mult)
                    nc.vector.reduce_sum(
                        out=tm3[:, 0:nt, 0:1], in_=tm3[:, 0:nt, :],
                        axis=mybir.AxisListType.X)
                    nc.vector.tensor_tensor(
                        out=al[:, 0:nt, 4:5], in0=al[:, 0:nt, 4:5],
                        in1=tm3[:, 0:nt, 0:1], op=OP.add)
                    # leaky relu + exp on Act engine
                    alp = wp.tile([P, TMAX, 5], F32, tag="alp")
                    nc.scalar.activation(alp[:, 0:nt, :], al[:, 0:nt, :],
                                         AF.Prelu, alpha=NEG)
                    rhsA = wp.tile([P, TMAX, 12], BF16, tag="rhsA")
                    nc.scalar.activation(rhsA[:, 0:nt, 0:5], alp[:, 0:nt, :], AF.Exp)
                    # messages: ex1*x0s (4), exd*xds (3)
                    nc.vector.tensor_tensor(
                        out=rhsA[:, 0:nt, 5:9],
                        in0=rhsA[:, 0:nt, 0:4],
                        in1=brd(xs, [xs.ap[0], [8, nt], [0, 4]]),
                        op=OP.mult)
                    nc.vector.tensor_tensor(
                        out=rhsA[:, 0:nt, 9:12],
                        in0=brd(xs, [xs.ap[0], [8, nt], [1, 3]], offset=xs.offset + 1),
                        in1=brd(rhsA[:], [rhsA[:].ap[0], [12, nt], [0, 3]],
                                offset=rhsA[:].offset + 4),
                        op=OP.mult)
                    psA = ppa.tile([P, 12], F32, tag="psA", space="PSUM")
                    for j in range(nt):
                        nc.tensor.matmul(
                            out=psA[:], lhsT=ohb_w[:, j, 0:P], rhs=rhsA[:, j, :],
                            start=(j == 0), stop=(j == nt - 1))
                    # epilogue A: r = num/den
                    den = wp.tile([P, 5], F32, tag="den")
                    nc.vector.tensor_scalar(
                        out=den[:], in0=psA[:, 0:5], scalar1=EPS, scalar2=None,
                        op0=OP.add)
                    nc.vector.reciprocal(out=den[:], in_=den[:])
                    nc.vector.tensor_tensor(
                        out=rA[:, w, 0:4], in0=psA[:, 5:9], in1=den[:, 0:4],
                        op=OP.mult)
                    nc.vector.tensor_tensor(
                        out=rA[:, w, 4:7], in0=psA[:, 9:12],
                        in1=den[:, 4:5].to_broadcast([P, 3]), op=OP.mult)

                    # ---- node phase for window w ----
                    h1 = npl.tile([P, HC], F32, tag="h1")
                    nc.vector.tensor_tensor(
                        out=brd(h1[:], [h1[:].ap[0], [C, H], [1, C]]),
                        in0=brd(w1_rep[:], [w1_rep[:].ap[0], [C, H], [1, C]]),
                        in1=brd(rA[:], [rA[:].ap[0], [1, H], [0, C]],
                                offset=rA[:].offset + w * 2 * H),
                        op=OP.mult)
                    nc.vector.tensor_tensor(
                        out=h1[:], in0=h1[:], in1=bs1_rep[:], op=OP.add)
                    # elu: relu(h1) + exp(min(h1,0)) - 1
                    rel = npl.tile([P, HC], F32, tag="rel")
                    nc.scalar.activation(rel[:], h1[:], AF.Relu)
                    nc.vector.tensor_tensor(
                        out=h1[:], in0=h1[:], in1=rel[:], op=OP.subtract)
                    nc.scalar.activation(h1[:], h1[:], AF.Exp)
                    h1e = npl.tile([P, HC], BF16, tag="h1e")
                    nc.vector.scalar_tensor_tensor(
                        out=h1e[:], in0=h1[:], scalar=-1.0, in1=rel[:],
                        op0=OP.add, op1=OP.add)
                    # h2 = h1e @ [ws2 | a-dots]  (via 2 transposed chunks)
                    h1t = npl.tile([P, 2, P], BF16, tag="h1t")
                    for ch in range(2):
                        pst = ppt.tile([P, P], BF16, tag="tr", space="PSUM", bufs=2)
                        nc.tensor.transpose(
                            out=pst[:], in_=h1e[:, ch * P:(ch + 1) * P],
                            identity=ident_bf[:])
                        nc.vector.tensor_copy(out=h1t[:, ch, :], in_=pst[:])
                    ph2 = ppm.tile([P, HC + 2 * H], F32, tag="mm", space="PSUM")
                    for ch in range(2):
                        nc.tensor.matmul(
                            out=ph2[:], lhsT=h1t[:, ch, :], rhs=ws2a_sb[:, ch, :],
                            start=(ch == 0), stop=(ch == 1))
                    h2row = npl.tile([P, HR], BF16, tag="h2row")
                    nc.vector.memset(h2row[:, HC + H:HR], 0.0)
                    nc.vector.tensor_copy(
                        out=h2row[:, 0:HC + H], in_=ph2[:, 0:HC + H])
                    nc.vector.tensor_copy(
                        out=sd2_all[:, w, :], in_=ph2[:, HC + H:HC + 2 * H])
                    nc.sync.dma_start(
                        out=h2slice[w * P:(w + 1) * P, :], in_=h2row[:])
                    # dynamic head: hd = rd @ Wd + bd
                    prd = ppt.tile([P, P], F32, tag="trf", space="PSUM")
                    nc.tensor.transpose(
                        out=prd[0:3, :], in_=rA[:, w, 4:7], identity=ident[:])
                    rdt = npl.tile([3, P], BF16, tag="rdt")
                    nc.vector.tensor_copy(out=rdt[:], in_=prd[0:3, :])
                    phd = ppm.tile([P, C], F32, tag="mmd", space="PSUM")
                    nc.tensor.matmul(
                        out=phd[:], lhsT=rdt[:], rhs=wd_sb[:], start=True, stop=True)
                    nc.vector.tensor_tensor(
                        out=hd_sb[:, w, :], in0=phd[:], in1=bd_rep[:], op=OP.add)
                    t0 += nt

            # ---------------- allgather ---------------------------------
            nc.gpsimd.collective_compute(
                "AllGather", OP.bypass,
                replica_groups=[list(range(W))],
                ins=[h2slice[:]], outs=[H2ext[:]])

            # ---------------- pass B ------------------------------------
            with (
                tc.tile_pool(name="ohB", bufs=3) as ohp,
                tc.tile_pool(name="hgB", bufs=2) as hgp,
                tc.tile_pool(name="wkB", bufs=2) as wp,
                tc.tile_pool(name="psB", bufs=2, space="PSUM") as ppb,
                tc.tile_pool(name="psS", bufs=2, space="PSUM") as pps,
            ):
                t0 = 0
                for w in range(nwin):
                    nt_lo = int(tpw[w, 0])
                    nt_hi = int(tpw[w, 1])
                    nt = nt_lo + nt_hi
                    ohb_w = ohp.tile([P, TMAX, 256], BF16, tag="ohb")
                    nc.sync.dma_start(
                        out=ohb_w[:, 0:nt, :], in_=ohb_in[:, t0:t0 + nt, :])
                    hg = hgp.tile([P, TMAX, HR], BF16, tag="hg")
                    GCAP = 8  # ring holds 1024 descriptors; cap idx per call
                    for (tb, ncall, base) in ((t0, nt_lo, 0),
                                              (t0 + nt_lo, nt_hi, plan.HALF)):
                        for q0 in range(0, ncall, GCAP):
                            qn = min(GCAP, ncall - q0)
                            ts = tb + q0
                            nc.gpsimd.dma_gather(
                                out_ap=hg[:, ts - t0:ts - t0 + qn, :],
                                in_ap=H2ext[base:base + plan.HALF, :],
                                idxs_ap=idx_sb[:, ts * 8:(ts + qn) * 8],
                                num_idxs=qn * P, num_idxs_reg=qn * P,
                                elem_size=HR)
                    # dst attn values broadcast: s2d_e = ohT @ sd2_all[w]
                    s2d = pps.tile([P, TMAX * H], F32, tag="s2d", space="PSUM")
                    for j in range(nt):
                        nc.tensor.matmul(
                            out=s2d[:, j * H:(j + 1) * H],
                            lhsT=ohb_w[:, j, P:2 * P], rhs=sd2_all[:, w, :],
                            start=True, stop=True)
                    al2 = wp.tile([P, TMAX, H], F32, tag="al2")
                    nc.vector.tensor_tensor(
                        out=al2[:, 0:nt, :],
                        in0=hg[:, 0:nt, HC:HC + H],
                        in1=brd(s2d[:], [s2d[:].ap[0], [H, nt], [1, H]]),
                        op=OP.add)
                    nc.vector.tensor_tensor(
                        out=al2[:, 0:nt, :], in0=al2[:, 0:nt, :],
                        in1=alE[:, t0:t0 + nt, 4:8], op=OP.add)
                    al2p = wp.tile([P, TMAX, H], F32, tag="al2p")
                    nc.scalar.activation(al2p[:, 0:nt, :], al2[:, 0:nt, :],
                                         AF.Prelu, alpha=NEG)
                    rhsB = wp.tile([P, TMAX, HC + H], BF16, tag="rhsB")
                    nc.scalar.activation(
                        rhsB[:, 0:nt, HC:HC + H], al2p[:, 0:nt, :], AF.Exp)
                    # msgs = hg * ex (heads 0-2 on DVE, head 3 on gpsimd)
                    nc.vector.tensor_tensor(
                        out=brd(rhsB[:], [rhsB[:].ap[0], [HC + H, nt], [C, H], [1, C]]),
                        in0=brd(hg[:], [hg[:].ap[0], [HR, nt], [C, H], [1, C]]),
                        in1=brd(rhsB[:], [rhsB[:].ap[0], [HC + H, nt], [1, H], [0, C]],
                                offset=rhsB[:].offset + HC),
                        op=OP.mult)
                    psB = ppb.tile([P, HC + H], F32, tag="psB", space="PSUM")
                    for j in range(nt):
                        nc.tensor.matmul(
                            out=psB[:], lhsT=ohb_w[:, j, 0:P], rhs=rhsB[:, j, :],
                            start=(j == 0), stop=(j == nt - 1))
                    # epilogue B
                    dn2 = wp.tile([P, H], F32, tag="dn2")
                    nc.vector.tensor_scalar(
                        out=dn2[:], in0=psB[:, HC:HC + H], scalar1=EPS,
                        scalar2=None, op0=OP.add)
                    nc.vector.reciprocal(out=dn2[:], in_=dn2[:])
                    agg = wp.tile([P, HC], F32, tag="agg")
                    nc.vector.tensor_tensor(
                        out=brd(agg[:], [agg[:].ap[0], [C, H], [1, C]]),
                        in0=brd(psB[:], [psB[:].ap[0], [C, H], [1, C]]),
                        in1=brd(dn2[:], [dn2[:].ap[0], [1, H], [0, C]]),
                        op=OP.mult)
                    hf = wp.tile([P, C], F32, tag="hf")
                    nc.vector.reduce_sum(
                        out=hf[:],
                        in_=brd(agg[:], [agg[:].ap[0], [1, C], [C, H]]),
                        axis=mybir.AxisListType.X)
                    # h = 0.25*hf + bs2 + hd
                    nc.vector.scalar_tensor_tensor(
                        out=hf[:], in0=hf[:], scalar=0.25, in1=bs2_rep[:],
                        op0=OP.mult, op1=OP.add)
                    nc.vector.tensor_tensor(
                        out=h_sb[:, w, :], in0=hf[:], in1=hd_sb[:, w, :], op=OP.add)
                    t0 += nt

            # ---------------- pooling + value head -----------------------
            with (
                tc.tile_pool(name="wkP", bufs=2) as wp,
                tc.tile_pool(name="psP", bufs=2, space="PSUM") as ppp,
            ):
                pg = ppp.tile([gpc, C], F32, tag="pg", space="PSUM")
                for w in range(nwin):
                    nc.tensor.matmul(
                        out=pg[:], lhsT=pm_all[:, w, :], rhs=h_sb[:, w, :],
                        start=(w == 0), stop=(w == nwin - 1))
                g_sb = wp.tile([gpc, C], F32, tag="g_sb")
                nc.vector.tensor_copy(out=g_sb[:], in_=pg[:])
                pgt = ppp.tile([C, gpc], F32, tag="tr", space="PSUM")
                nc.tensor.transpose(
                    out=pgt[:], in_=g_sb[:], identity=ident[0:gpc, 0:gpc])
                gt_sb = wp.tile([C, gpc], F32, tag="gt_sb")
                nc.vector.tensor_copy(out=gt_sb[:], in_=pgt[:])
                pv1 = ppp.tile([gpc, C], F32, tag="mm", space="PSUM")
                nc.tensor.matmul(
                    out=pv1[:], lhsT=gt_sb[:], rhs=wv1_sb[:], start=True, stop=True)
                a_sb = wp.tile([gpc, C], F32, tag="a_sb")
                nc.vector.tensor_tensor(
                    out=a_sb[:], in0=pv1[:], in1=bv1_rep[0:gpc, :], op=OP.add)
                nc.vector.tensor_scalar(
                    out=a_sb[:], in0=a_sb[:], scalar1=0.0, scalar2=None, op0=OP.max)
                pat = ppp.tile([C, gpc], F32, tag="tr", space="PSUM")
                nc.tensor.transpose(
                    out=pat[:], in_=a_sb[:], identity=ident[0:gpc, 0:gpc])
                at_sb = wp.tile([C, gpc], F32, tag="at_sb")
                nc.vector.tensor_copy(out=at_sb[:], in_=pat[:])
                pv2 = ppp.tile([gpc, 1], F32, tag="mm2", space="PSUM")
                nc.tensor.matmul(
                    out=pv2[:], lhsT=at_sb[:], rhs=wv2_sb[:], start=True, stop=True)
                vres = wp.tile([gpc, 1], F32, tag="vres")
                nc.vector.tensor_tensor(
                    out=vres[:], in0=pv2[:], in1=bv2_rep[0:gpc, :], op=OP.add)
                nc.sync.dma_start(out=v_out[:], in_=vres[:])

    nc.compile()
    return nc


# ----------------------------------------------------------------------------
# in_maps assembly
# ----------------------------------------------------------------------------

def make_in_maps(plan, per_core_arrays, weights):
    w = {k: np.ascontiguousarray(v, np.float32) for k, v in weights.items()}
    shared = dict(
        ws1=w["Ws1"].reshape(1, HC),
        a1s=w["as_src1"].reshape(1, HC),
        a1d=w["as_dst1"].reshape(1, HC),
        we1=w["We1"].reshape(1, 2 * HC),
        ae1=w["ae1"].reshape(1, HC),
        bs1=w["bs1"].reshape(1, HC),
        ws2=w["Ws2"],
        a2s=w["as_src2"].reshape(1, HC),
        a2d=w["as_dst2"].reshape(1, HC),
        we2=w["We2"].reshape(1, 2 * HC),
        ae2=w["ae2"].reshape(1, HC),
        bs2=w["bs2"].reshape(1, C),
        wd=w["Wd"],
        wdf=w["Wd"].reshape(1, 3 * C),
        ads=w["ad_src"].reshape(1, C),
        add=w["ad_dst"].reshape(1, C),
        bd=w["bd"].reshape(1, C),
        wv1=w["Wv1"],
        bv1=w["bv1"].reshape(1, C),
        wv2=w["Wv2"],
        bv2=w["bv2"].reshape(1, 1),
    )
    in_maps = []
    for c in range(plan.W):
        m = dict(shared)
        m.update(per_core_arrays[c])
        in_maps.append(m)
    return in_maps


_CACHE = {}


def prepare(inputs):
    x = np.asarray(inputs["x"])
    edge_attr = np.asarray(inputs["edge_attr"])
    edge_index = np.asarray(inputs["edge_index"])
    batch = np.asarray(inputs["batch"])
    G = 64
    W = 8
    plan, pca = host_prep(x, edge_attr, edge_index, batch, G, W)
    key = (plan.R, plan.T, tuple(plan.tpw.ravel()))
    if key not in _CACHE:
        _CACHE[key] = build_bass(plan)
    nc = _CACHE[key]
    weights = {k: inputs[k] for k in (
        "Ws1", "as_src1", "as_dst1", "We1", "ae1", "bs1",
        "Ws2", "as_src2", "as_dst2", "We2", "ae2", "bs2",
        "Wd", "ad_src", "ad_dst", "bd", "Wv1", "bv1", "Wv2", "bv2")}
    in_maps = make_in_maps(plan, pca, weights)
    return nc, in_maps, plan


def kernel(**inputs):
    nc, in_maps, plan = prepare(inputs)
    from concourse.bass_utils import run_bass_kernel_spmd
    res = run_bass_kernel_spmd(nc, in_maps, list(range(plan.W)))
    v = np.concatenate([res.results[c]["v"][:, 0] for c in range(plan.W)])
    return v.astype(np.float32)
